# revision 20
# baseline (speedup 1.0000x reference)
"""Trainium2 Bass kernel for per-sample-routed ConvTranspose1d (Dereverb T60
decoder).

Math: for each sample b with routed weight W (Cin=512, K=16), stride 8, pad 8:
    y[t] = A[p, q] + A[p+8, q-1]   where t + 8 = 8q + p (p in [0,8)),
    A[k, q] = sum_ci W[ci, k] * x[ci, q]        (a 16x512 @ 512x4000 matmul)
so t = 8m + p with m = q-1 in [0, 3999).

Sharding: pure data parallel, B=16 -> 2 samples on each of 8 NeuronCores.
Routing (t60 -> 1 of 41 kernels) is a host-side gather of 32KB per sample.

Device kernel per core, per sample:
  - lhsT "w40" (128, 4 chunks, 40): taps 0..7 at cols 0..7 (lo), taps 8..15 at
    cols 32..39 (hi) -- engine ops need base partitions in {0,32,64,96}.
  - 8 j-tiles: psum_A (40, 512) accumulates 4 K-chunk matmuls (fp32 exact).
  - copy lo rows to SBUF alo (8, 4000); add alo[:, m+1] + psum_hi[:, m]
    (mixed-space operands may differ in base partition) into z128, which
    stacks 4 blocks of 1024 m's at partition bases {0,32,64,96}.
  - 8 PE transposes (128,128) -> psum_t[j, 32A+p] = y[8*(1024A+128k+j)+p];
    copy valid cols into y_sb (128, 4A, 8k, 8p); 3 DMAs to HBM.
"""
import numpy as np

import concourse.bass as bass
import concourse.tile as tile
from concourse import bacc, mybir
from concourse.bass_utils import run_bass_kernel_spmd
from concourse.masks import make_identity

B, CIN, L, KSZ = 16, 512, 4000, 16
LOUT = (L - 1) * 8 - 2 * 8 + KSZ  # 31992
NCORES = 8
PER = B // NCORES                 # 2 samples per core
NCHUNK = CIN // 128               # 4
JW = 512
NJ = 8                            # ceil(4000/512)
MV = L - 1                        # 3999 valid output m positions
F32 = mybir.dt.float32

_CACHE = {}


def _build(reps=1, f32r=False, xslices=8, mode="full"):
    # x load granularity: bounds of the per-sample column slices (512-aligned)
    if isinstance(xslices, (list, tuple)):
        xbounds = list(xslices)
    elif xslices == 1:
        xbounds = [0, L]
    elif xslices == 2:
        xbounds = [0, 2048, L]
    elif xslices == 4:
        xbounds = [0, 1024, 2048, 3072, L]
    else:
        xbounds = [JW * j for j in range(NJ)] + [L]
    xw_max = max(b - a for a, b in zip(xbounds[:-1], xbounds[1:]))
    nc = bacc.Bacc("TRN2", target_bir_lowering=False, debug=False,
                   num_devices=NCORES)
    x = nc.dram_tensor("x", [PER, CIN, L], F32, kind="ExternalInput").ap()
    w = nc.dram_tensor("w", [PER, CIN, KSZ], F32, kind="ExternalInput").ap()
    y = nc.dram_tensor("y", [PER, LOUT], F32, kind="ExternalOutput").ap()

    with tile.TileContext(nc) as tc:
        xbufs = 4 if xw_max <= 1536 else (3 if xw_max <= 2048 else 2)
        with tc.tile_pool(name="xp", bufs=xbufs) as xp, \
             tc.tile_pool(name="wp", bufs=2) as wp, \
             tc.tile_pool(name="alop", bufs=2) as alop, \
             tc.tile_pool(name="zp", bufs=2) as zp, \
             tc.tile_pool(name="yp", bufs=2) as yp, \
             tc.tile_pool(name="cst", bufs=1) as cst, \
             tc.tile_pool(name="pa", bufs=4, space="PSUM") as pa, \
             tc.tile_pool(name="pt", bufs=2, space="PSUM") as pt:

            ident = cst.tile([128, 128], F32)
            make_identity(nc, ident[:])

            def mmdt(ap):
                return ap.bitcast(mybir.dt.float32r) if f32r else ap

            xts_shared = None
            if mode == "lsprobe":
                # measure fp32 self-loading-weights matmul rate: B-shaped
                # stream -- per sample 32 q-tiles x 8 accumulating matmuls
                # with lhsT = (128,128) slice of x, rhs = (128,8) weights
                xt0 = xp.tile([128, NCHUNK, xw_max], F32, tag="xt")
                nc.sync.dma_start(
                    xt0[:, :, 0:xbounds[1]],
                    x[0].rearrange("(c p) l -> p c l", p=128)
                       [:, :, 0:xbounds[1]])
                w40p = wp.tile([128, NCHUNK, 40], F32, tag="w40")
                nc.vector.memset(w40p[:], 0.0)
                zt = cst.tile([128, 250], F32, tag="zt")
                nc.vector.memset(zt[:], 0.0)
                for rep in range(reps):
                    for s in range(PER):
                        for t in range(32):
                            pz = pa.tile([128, 8], F32, tag="pz")
                            for g in range(8):
                                nc.tensor.matmul(
                                    pz[:], xt0[:, g % NCHUNK,
                                               (g % 2): (g % 2) + 128],
                                    w40p[:, 0, 0:8],
                                    start=(g == 0), stop=(g == 7))
                        nc.sync.dma_start(
                            y[s][0:31872].rearrange("(j f) -> j f",
                                                    j=128, f=249),
                            zt[:, 0:249])
            if mode == "noxdma":
                xts_shared = []
                for a, b in zip(xbounds[:-1], xbounds[1:]):
                    xt = xp.tile([128, NCHUNK, xw_max], F32, tag="xt")
                    nc.sync.dma_start(
                        xt[:, :, 0:b - a],
                        x[0].rearrange("(c p) l -> p c l", p=128)[:, :, a:b])
                    xts_shared.append((xt, a))

            sample_seq = ([] if mode == "lsprobe" else
                          [s for _ in range(reps) for s in range(PER)])
            for s in sample_seq:
                w40 = wp.tile([128, NCHUNK, 40], F32, tag="w40")
                nc.vector.memset(w40[:], 0.0)
                wr = w[s].rearrange("(c p) k -> p c k", p=128)
                nc.sync.dma_start(w40[:, :, 0:8], wr[:, :, 0:8])
                nc.sync.dma_start(w40[:, :, 32:40], wr[:, :, 8:16])

                alo = alop.tile([8, L], F32, tag="alo")
                xts = [] if xts_shared is None else xts_shared
                psums = []
                for j in range(NJ):
                    n = min(JW, L - JW * j)  # 512 or 416
                    j0 = JW * j
                    if len(xts) < len(xbounds) - 1 and j0 >= xbounds[len(xts)]:
                        a, b = xbounds[len(xts)], xbounds[len(xts) + 1]
                        xt = xp.tile([128, NCHUNK, xw_max], F32, tag="xt")
                        nc.sync.dma_start(
                            xt[:, :, 0:b - a],
                            x[s].rearrange("(c p) l -> p c l", p=128)
                               [:, :, a:b])
                        xts.append((xt, a))
                    if mode == "dmaonly":
                        continue
                    xt, a = next((t, a) for t, a in reversed(xts)
                                 if a <= j0)
                    ps = pa.tile([40, JW], F32, tag="pa")
                    psums.append(ps)
                    for c in range(NCHUNK):
                        nc.tensor.matmul(ps[:, 0:n], mmdt(w40[:, c, :]),
                                         mmdt(xt[:, c, j0 - a: j0 - a + n]),
                                         start=(c == 0),
                                         stop=(c == NCHUNK - 1))
                    nc.vector.tensor_copy(alo[:, j0: j0 + n],
                                          ps[0:8, 0:n])

                if mode == "dmaonly":
                    zt = cst.tile([128, 250], F32, tag="zt")
                    nc.vector.memset(zt[:], 0.0)
                    nc.sync.dma_start(
                        y[s][0:31872].rearrange("(j f) -> j f", j=128, f=249),
                        zt[:, 0:249])
                    nc.sync.dma_start(
                        y[s][31872:31992].rearrange("(j f) -> j f",
                                                    j=120, f=1),
                        zt[0:120, 0:1])
                    continue

                z128 = zp.tile([128, 1024], F32, tag="z128")
                nc.vector.memset(z128[:], 0.0)
                for a in range(4):
                    for h in range(2):
                        j = 2 * a + h
                        m0 = 1024 * a + 512 * h
                        n = min(512, MV - m0)  # 512, last piece 415
                        nc.vector.tensor_tensor(
                            z128[32 * a: 32 * a + 8, 512 * h: 512 * h + n],
                            alo[0:8, m0 + 1: m0 + 1 + n],
                            psums[j][32:40, 0:n],
                            mybir.AluOpType.add)

                ysb = yp.tile([128, 4, 8, 8], F32, tag="ysb")
                for k in range(8):
                    tps = pt.tile([128, 128], F32, tag="pt")
                    nc.tensor.matmul(tps[:], z128[:, 128 * k: 128 * (k + 1)],
                                     ident[:], is_transpose=True,
                                     start=True, stop=True)
                    nc.vector.tensor_copy(
                        ysb[:, :, k, :],
                        tps[:].rearrange("j (a q) -> j a q", a=4)[:, :, 0:8])

                # y[8*(1024A + 128k + j) + p] = ysb[j, A, k, p]
                nc.sync.dma_start(
                    y[s][0:24576].rearrange("(a k j p) -> j a k p",
                                            a=3, k=8, j=128, p=8),
                    ysb[:, 0:3, :, :])
                nc.sync.dma_start(
                    y[s][24576:31744].rearrange("(k j p) -> j k p",
                                                k=7, j=128, p=8),
                    ysb[:, 3, 0:7, :])
                nc.sync.dma_start(
                    y[s][31744:31992].rearrange("(j p) -> j p", j=31, p=8),
                    ysb[0:31, 3, 7, :])

    nc.compile()
    return nc


def _route(t60s):
    idx = np.round(t60s.astype(np.float32) * np.float32(100.0))
    return np.tile(idx.astype(np.int32), 2) - 10  # (B,)


XBOUNDS_DEFAULT = (0, 512, 1536, 2560, 4000)


def get_nc(reps=1, f32r=False):
    import os
    xs_env = os.environ.get("XSLICES", "a")
    xslices = XBOUNDS_DEFAULT if xs_env == "a" else int(xs_env)
    key = (reps, f32r, tuple(xslices) if isinstance(xslices, tuple) else xslices)
    if key not in _CACHE:
        _CACHE[key] = _build(reps=reps, f32r=f32r, xslices=xslices)
    return _CACHE[key]


def make_in_maps(input, t60s, kernel_weight):
    idx = _route(np.asarray(t60s))
    wg = np.asarray(kernel_weight)[idx, :, 0, :]  # (B, Cin, K)
    xin = np.asarray(input, dtype=np.float32)
    in_maps = []
    for c in range(NCORES):
        sl = slice(PER * c, PER * (c + 1))
        in_maps.append({
            "x": np.ascontiguousarray(xin[sl]),
            "w": np.ascontiguousarray(wg[sl].astype(np.float32)),
        })
    return in_maps


def _run(input, t60s, kernel_weight, trace=False):
    nc = get_nc()

    in_maps = make_in_maps(input, t60s, kernel_weight)
    res = run_bass_kernel_spmd(nc, in_maps, core_ids=list(range(NCORES)),
                               trace=trace)
    out = np.empty((B, 1, LOUT), dtype=np.float32)
    for c in range(NCORES):
        out[PER * c: PER * (c + 1), 0, :] = res.results[c]["y"]
    return out, res


def kernel(input, t60s, kernel_weight):
    out, _ = _run(input, t60s, kernel_weight, trace=False)
    return out


def kernel_traced(input, t60s, kernel_weight):
    out, res = _run(input, t60s, kernel_weight, trace=True)
    return out, res
